# revision 2
# baseline (speedup 1.0000x reference)
"""Trainium2 Bass kernel for nn_CBPoolMax2d — sparse gather version.

Reference semantics: changeIndexes are flat spatial indices (y*W+x) of changed
input pixels; each maps to output pixel (y//2, x//2).  The output is the
persistent outputState with the 2x2-max-pooled value recomputed at every
changed output pixel (all channels).

Sparse formulation: only ~25.8K of the 65536 output pixels are touched
(K=32768 random indices, 39% distinct-window density).  Instead of streaming
all 256 MB of input through the cores (the dense roofline, ~144 us), gather
just the changed 2x2 windows:

  host:   input [1,C,H,W] f32 -> window-major channels-last bf16
          [OH, OW, 2, 2, C]: each output pixel's 2x2xC window is one
          contiguous 2 KB element.  Shard by output-row band (32 rows/core).
  device: gpsimd.dma_gather pulls the changed windows, DVE reduces 4->1 with
          two bf16 max stages (2x_1p fast mode), dense bf16 writeback.
  host:   out = outputState.copy(); out[:, oy, ox] = pooled.T  (unshard).

The bottleneck is Q7 SWDGE descriptor generation (~7.5 ns/idx), so adjacent
changed windows are coalesced: ox-groups of 4 with >=3 hits become one 8 KB
quad element, remaining hits become aligned-pair (4 KB) or single (2 KB)
elements — ~2.3K descriptors/core instead of 3.2K, balancing Q7 against DMA
bytes.  The idx DMA rides the sync engine under the mlp library-load shadow;
chunks taper so the last exposed transfer is short; the block skips the
gpsimd dge_drain (all gather DMAs are provably complete via gsem).
"""

import numpy as np
import ml_dtypes

C, H, W = 256, 512, 512
OH, OW = H // 2, W // 2
NCORES = 8
BAND = OH // NCORES            # 32 output rows per core
NROWS = BAND * OW              # 8192 single elements per core band
ELEM = 4 * C                   # 1024 bf16 values = 2 KB single element
NB = 4                         # G buffer count
KT = 3                         # quad threshold: >=KT hits in an ox-group of 4

TRACE = __import__("os").environ.get("CBPOOL_TRACE", "0") == "1"
last_results = None

_cache = {}


def _chunk_list(slots, big, taper):
    """Split `slots` into chunks of at most `big`; if `taper`, end with a
    short chunk so the last gather's exposed DMA transfer is small."""
    out = []
    rem = slots
    while rem > big:
        out.append(big)
        rem -= big
    if taper and rem > 2:
        out.extend([rem - 2, 2])
    elif rem:
        out.append(rem)
    return out


def _build_nc(pad_q, pad_p, pad_s):
    import concourse.bacc as bacc
    from concourse import bass, mybir
    from concourse.library_config import mlp
    from contextlib import ExitStack

    bf16 = mybir.dt.bfloat16
    u8 = mybir.dt.uint8
    i16 = mybir.dt.int16
    S_Q, S_P, S_S = pad_q // 128, pad_p // 128, pad_s // 128
    idx_cols = (pad_q + pad_p + pad_s) // 16

    nc = bacc.Bacc("TRN2", target_bir_lowering=False, debug=False,
                   num_devices=NCORES)
    srcp = nc.dram_tensor("srcp", [NROWS // 2, 2 * ELEM], bf16,
                          kind="ExternalInput")
    idxt = nc.dram_tensor("idx", [128, idx_cols], i16, kind="ExternalInput")
    outq = nc.dram_tensor("outq", [128, max(S_Q, 1), 4 * C], bf16,
                          kind="ExternalOutput")
    outp = nc.dram_tensor("outp", [128, max(S_P, 1), 2 * C], bf16,
                          kind="ExternalOutput")
    outs = nc.dram_tensor("outs", [128, max(S_S, 1), C], bf16,
                          kind="ExternalOutput")
    # same DRAM buffer viewed at quad/pair/single element granularity
    src_w = {
        4: bass.AP(srcp, 0, [[4 * ELEM, NROWS // 4], [1, 4 * ELEM]]),
        2: srcp[:],
        1: bass.AP(srcp, 0, [[ELEM, NROWS], [1, ELEM]]),
    }
    out_w = {4: outq, 2: outp, 1: outs}

    # chunk plan: [(w, class_slot_off, cs)] — quads, pairs, then singles
    plan = []
    for wc, slots, big, taper in ((4, S_Q, 4, False), (2, S_P, 6, False),
                                  (1, S_S, 8, True)):
        off = 0
        for cs in _chunk_list(slots, big, taper):
            plan.append((wc, off, cs))
            off += cs
    n_chunks = len(plan)
    g_cap = max(cs * wc * ELEM for wc, o, cs in plan)

    with ExitStack() as st:
        blk = st.enter_context(nc.Block(no_gpsimd_drain=True))
        idx_sb = st.enter_context(nc.sbuf_tensor("idx_sb", [128, idx_cols], i16))
        G = [st.enter_context(nc.sbuf_tensor(f"g{i}", [128, g_cap], bf16))
             for i in range(NB)]
        M = [st.enter_context(nc.sbuf_tensor(f"m{i}", [128, g_cap // 2], bf16))
             for i in range(2)]
        V = [st.enter_context(nc.sbuf_tensor(f"v{i}", [128, g_cap // 4], bf16))
             for i in range(2)]
        isem = st.enter_context(nc.semaphore("isem"))
        gsems = [st.enter_context(nc.semaphore(f"gsem{i}"))
                 for i in range(NB)]
        s1sem = st.enter_context(nc.semaphore("s1sem"))
        vsem = st.enter_context(nc.semaphore("vsem"))
        wsems = [st.enter_context(nc.semaphore(f"wsem{i}"))
                 for i in range(2)]

        @blk.sync
        def _(sy):
            # idx upload runs under the mlp library-load shadow
            sy.dma_start(idx_sb[:], idxt[:]).then_inc(isem, 16)

        @blk.gpsimd
        def _(gp):
            gp.load_library(mlp)
            gp.wait_ge(isem, 16)
            icol = 0
            for c, (wc, soff, cs) in enumerate(plan):
                n_idx = 128 * cs
                if c >= NB:
                    # G[c%NB] free once stage-1 of chunk c-NB has run
                    gp.wait_ge(s1sem, c - NB + 1)
                gp.dma_gather(
                    G[c % NB][:, :cs * wc * ELEM].rearrange(
                        "p (s e) -> p s e", s=cs, e=wc * ELEM),
                    src_w[wc],
                    idx_sb[:, icol: icol + n_idx // 16],
                    n_idx, n_idx, wc * ELEM,
                ).then_inc(gsems[c % NB], 16)
                icol += n_idx // 16

        @blk.vector
        def _(ve):
            for c, (wc, soff, cs) in enumerate(plan):
                # per-buffer sem: immune to cross-chunk DMA-engine reordering
                ve.wait_ge(gsems[c % NB], 16 * (c // NB + 1))
                # stage 1: max over the two rows of each window
                g5 = G[c % NB][:, :cs * wc * ELEM].rearrange(
                    "p (s w r x) -> p s w r x", s=cs, w=wc, r=2, x=ELEM // 2)
                m5 = M[c % 2][:, :cs * wc * ELEM // 2].rearrange(
                    "p (s w x) -> p s w x", s=cs, w=wc, x=ELEM // 2)
                ve.tensor_tensor(out=m5, in0=g5[:, :, :, 0, :],
                                 in1=g5[:, :, :, 1, :],
                                 op=mybir.AluOpType.max).then_inc(s1sem, 1)
                if c >= 2:
                    # V[c%2] free once writeback of chunk c-2 has completed
                    ve.wait_ge(wsems[c % 2], 16 * ((c - 2) // 2 + 1))
                # stage 2: max over the two columns
                m6 = M[c % 2][:, :cs * wc * ELEM // 2].rearrange(
                    "p (s w cl x) -> p s w cl x", s=cs, w=wc, cl=2, x=C)
                v5 = V[c % 2][:, :cs * wc * C].rearrange(
                    "p (s w x) -> p s w x", s=cs, w=wc, x=C)
                ve.tensor_tensor(out=v5, in0=m6[:, :, :, 0, :],
                                 in1=m6[:, :, :, 1, :],
                                 op=mybir.AluOpType.max).then_inc(vsem, 1)

        @blk.scalar
        def _(sc):
            for c, (wc, soff, cs) in enumerate(plan):
                sc.wait_ge(vsem, c + 1)
                sc.dma_start(
                    out_w[wc][:, soff: soff + cs, :],
                    V[c % 2][:, :cs * wc * C].rearrange(
                        "p (s x) -> p s x", s=cs, x=wc * C),
                ).then_inc(wsems[c % 2], 16)
            for k in range(2):
                tot = sum(1 for c in range(n_chunks) if c % 2 == k)
                if tot:
                    sc.wait_ge(wsems[k], 16 * tot)

    nc.compile()
    return nc


def _get_nc(pads):
    if pads not in _cache:
        _cache[pads] = _build_nc(*pads)
    return _cache[pads]


def _pad_wrap16(ids, pad_n):
    """int16 ids padded to pad_n, wrapped: idx i at partition i%16, col i//16,
    replicated over the 8 16-partition stripes."""
    pad = np.full(pad_n, ids[-1] if len(ids) else 0, dtype=np.int16)
    pad[:len(ids)] = ids
    return np.tile(pad.reshape(pad_n // 16, 16).T, (8, 1))


def kernel(input, outputState, changeIndexes):
    global last_results
    from concourse.bass_utils import run_bass_kernel_spmd

    inp = np.asarray(input, dtype=np.float32).reshape(C, H, W)
    state = np.asarray(outputState, dtype=np.float32).reshape(C, OH, OW)
    ci = np.asarray(changeIndexes).astype(np.int64)

    # distinct changed output pixels as a dense mask per band
    oyx = np.unique((ci // W) // 2 * OW + (ci % W) // 2)
    mask = np.zeros(OH * OW, dtype=bool)
    mask[oyx] = True
    mask = mask.reshape(OH, OW)

    # per-band quad/pair/single decomposition
    quad_ids, pair_ids, sing_ids = [], [], []
    for i in range(NCORES):
        bm = mask[i * BAND:(i + 1) * BAND]          # [BAND, OW]
        h4 = bm.reshape(BAND, OW // 4, 4)
        qm = h4.sum(-1) >= KT                       # quad groups
        qy, qt = np.nonzero(qm)
        quad_ids.append((qy * (OW // 4) + qt).astype(np.int16))
        rest = h4 & ~qm[:, :, None]
        rest = rest.reshape(BAND, OW)
        both = rest[:, 0::2] & rest[:, 1::2]        # aligned pair hits
        py, px = np.nonzero(both)
        pair_ids.append((py * (OW // 2) + px).astype(np.int16))
        sm = rest.copy()
        sm[:, 0::2] &= ~both
        sm[:, 1::2] &= ~both
        sy, sx = np.nonzero(sm)
        sing_ids.append((sy * OW + sx).astype(np.int16))

    pads = tuple(max(128, -(-max(len(x) for x in lst) // 128) * 128)
                 for lst in (quad_ids, pair_ids, sing_ids))

    # window-major channels-last bf16: [OH, OW, 2, 2, C]
    winp = np.ascontiguousarray(
        inp.reshape(C, OH, 2, OW, 2).transpose(1, 3, 2, 4, 0)
    ).astype(ml_dtypes.bfloat16)

    nc = _get_nc(pads)

    in_maps = []
    for i in range(NCORES):
        idx128 = np.concatenate(
            [_pad_wrap16(quad_ids[i], pads[0]),
             _pad_wrap16(pair_ids[i], pads[1]),
             _pad_wrap16(sing_ids[i], pads[2])], axis=1)
        in_maps.append({
            "srcp": winp[i * BAND:(i + 1) * BAND].reshape(NROWS // 2, 2 * ELEM),
            "idx": np.ascontiguousarray(idx128),
        })

    res = run_bass_kernel_spmd(nc, in_maps, core_ids=list(range(NCORES)),
                               trace=TRACE)
    last_results = res

    out_full = state.copy()
    for i in range(NCORES):
        r = res.results[i]
        bmask = mask[i * BAND:(i + 1) * BAND]
        for name, wc, ids in (("outq", 4, quad_ids[i]),
                              ("outp", 2, pair_ids[i]),
                              ("outs", 1, sing_ids[i])):
            n = len(ids)
            if n == 0:
                continue
            # v[j, k, :] = channels of window k of element j (id order)
            v = np.asarray(r[name]).astype(np.float32)
            v = v.transpose(1, 0, 2).reshape(-1, wc, C)[:n]
            il = ids.astype(np.int64)
            gy = il // (OW // wc)
            gx = (il % (OW // wc)) * wc
            for k in range(wc):
                oy, ox = gy, gx + k
                hit = bmask[oy, ox] if wc == 4 else np.ones(n, bool)
                out_full[:, i * BAND + oy[hit], ox[hit]] = v[hit, k, :].T
    return out_full.reshape(1, C, OH, OW)
